# revision 15
# baseline (speedup 1.0000x reference)
"""Bilateral filter (nn_BilateralFilter) Trainium2 Bass kernel.

Reference semantics (KERNEL_SIZE=5, THETA_ALPHA=2.0, THETA_BETA=0.1):
    w_k   = exp(-(dx^2+dy^2)/8)                      (24 offsets, center dropped)
    Ki    = exp(-50*(I(p+k) - I(p))^2)               per image channel c
    out[c,n,p] = sum_k w_k*Ki[c,k,p]*Q(n,p+k) / sum_k w_k*Ki[c,k,p]

Sharding: 8 cores = 2 batches x 4 row-slabs of 80 output rows, each slab
shipped with a 2-row halo and 2-col zero padding (84 x 324 per channel).
Device layout: partitions = image rows, free dim = (channel, column).
fp16 on-chip (DVE 2x mode); exponent computed via ACT (Square then Exp with
the spatial weight folded into the exp bias, plus a +8 exponent shift to
keep fp16 sums well inside normal range; the shift cancels in the final
division).  A column-shifted copy of I and Q keeps every access 4-byte
aligned so the DVE stays in its accelerated mode for odd column offsets.
Compute-engine SBUF accesses spanning >32 partitions must start at
partition 0, so each row shift dr gets its own 80-partition copy (5 blocks
packed in one tile, one DMA each for I/Ishift/Q/Qshift).
"""

import math

import numpy as np

B, C, NCL = 2, 3, 6
H = W = 320
KS, PAD = 5, 2
NK = KS * KS - 1          # 24
WP = W + 2 * PAD          # 324
NSLAB = 4
R = H // NSLAB            # 80 output rows per shard
RH = R + 2 * PAD          # 84 rows incl. halo
COEF = 50.0               # 1/(2*theta_beta^2)
SHIFT = 8.0               # exponent shift, cancels in the division
IW = C * WP               # 972
QW = NCL * WP             # 1944

_CACHE: dict = {}


def _offsets():
    return [
        (dr, dc)
        for dr in range(KS)
        for dc in range(KS)
        if not (dr == PAD and dc == PAD)
    ]


def _emit(tc, i_ap, q_ap, out_ap):
    """Emit the per-core program into TileContext tc.

    i_ap:   DRAM AP (RH, C*WP)  fp16
    q_ap:   DRAM AP (RH, NCL*WP) fp16
    out_ap: DRAM AP (R, C*NCL*W) fp16
    """
    import concourse.bass as bass
    import concourse.mybir as mybir

    f16 = mybir.dt.float16
    f32 = mybir.dt.float32
    AF = mybir.ActivationFunctionType
    nc = tc.nc
    offs = _offsets()

    with tc.tile_pool(name="p", bufs=1) as pool:
        # distinct exp biases: SHIFT - (dx^2+dy^2)/8 (+ln of nothing; w in bias)
        bvals = sorted({(dr - PAD) ** 2 + (dc - PAD) ** 2 for dr, dc in offs})
        bias_col = {v: j for j, v in enumerate(bvals)}
        bias_t = pool.tile([R, len(bvals)], f32, tag="bias")
        for v, j in bias_col.items():
            nc.vector.memset(bias_t[:, j : j + 1], SHIFT - v / 8.0)

        Ia = pool.tile([R, KS * IW], f16, tag="Ia")
        Ib = pool.tile([R, KS * IW], f16, tag="Ib")
        Qa = pool.tile([R, KS * QW], f16, tag="Qa")
        Qb = pool.tile([R, KS * QW], f16, tag="Qb")

        nc.sync.dma_start(
            Ia[:, :].rearrange("p (dr w) -> p dr w", dr=KS),
            bass.AP(tensor=i_ap.tensor, offset=0,
                    ap=[[IW, R], [IW, KS], [1, IW]]),
        )
        nc.sync.dma_start(
            Ib[:, :].rearrange("p (dr c w) -> p dr c w", dr=KS, c=C)[
                :, :, :, : WP - 1
            ],
            bass.AP(tensor=i_ap.tensor, offset=1,
                    ap=[[IW, R], [IW, KS], [WP, C], [1, WP - 1]]),
        )
        nc.sync.dma_start(
            Qa[:, :].rearrange("p (dr w) -> p dr w", dr=KS),
            bass.AP(tensor=q_ap.tensor, offset=0,
                    ap=[[QW, R], [QW, KS], [1, QW]]),
        )
        nc.sync.dma_start(
            Qb[:, :].rearrange("p (dr n w) -> p dr n w", dr=KS, n=NCL)[
                :, :, :, : WP - 1
            ],
            bass.AP(tensor=q_ap.tensor, offset=1,
                    ap=[[QW, R], [QW, KS], [WP, NCL], [1, WP - 1]]),
        )

        def ia_v(dr, c, dc):
            if dc % 2 == 0:
                return Ia[:, dr * IW + c * WP + dc : dr * IW + c * WP + dc + W]
            return Ib[:, dr * IW + c * WP + dc - 1 : dr * IW + c * WP + dc - 1 + W]

        def qa_v(dr, dc):
            if dc % 2 == 0:
                src, off = Qa, dc
            else:
                src, off = Qb, dc - 1
            return src[:, dr * QW : (dr + 1) * QW].rearrange(
                "p (n w) -> p n w", n=NCL
            )[:, :, off : off + W]

        d_t, sq_t, kw_t = [], [], []
        for c in range(C):
            d = pool.tile([R, NK * W], f16, tag=f"d{c}")
            cen = ia_v(PAD, c, PAD)
            for ki, (dr, dc) in enumerate(offs):
                nc.vector.tensor_sub(
                    d[:, ki * W : (ki + 1) * W], ia_v(dr, c, dc), cen
                )
            sq = pool.tile([R, NK * W], f16, tag=f"sq{c}")
            nc.scalar.activation(sq[:, :], d[:, :], AF.Square)
            d_t.append(d)
            sq_t.append(sq)

        for c in range(C):
            # kw reuses d's SBUF slot (d is dead once sq is computed)
            kw = pool.tile([R, NK * W], f16, tag=f"d{c}")
            kw_t.append(kw)
            for ki, (dr, dc) in enumerate(offs):
                v = (dr - PAD) ** 2 + (dc - PAD) ** 2
                nc.scalar.activation(
                    kw[:, ki * W : (ki + 1) * W],
                    sq_t[c][:, ki * W : (ki + 1) * W],
                    AF.Exp,
                    bias=bias_t[:, bias_col[v] : bias_col[v] + 1],
                    scale=-COEF,
                )

        for c in range(C):
            kw = kw_t[c]
            acc = pool.tile([R, NCL * W], f16, tag=f"acc{c}")
            accv = acc[:, :].rearrange("p (n w) -> p n w", n=NCL)
            for ki, (dr, dc) in enumerate(offs):
                kwb = (
                    kw[:, ki * W : (ki + 1) * W]
                    .unsqueeze(1)
                    .broadcast_to([R, NCL, W])
                )
                qv = qa_v(dr, dc)
                if ki == 0:
                    nc.vector.tensor_mul(accv, kwb, qv)
                else:
                    P = pool.tile([R, NCL * W], f16, tag="P", bufs=2)
                    Pv = P[:, :].rearrange("p (n w) -> p n w", n=NCL)
                    nc.vector.tensor_mul(Pv, kwb, qv)
                    nc.vector.tensor_add(accv, accv, Pv)

            # norm = sum_k kw  (pairwise tree: 24 -> 12 -> 6 -> 3 -> 1)
            # t12 reuses sq's (now dead) SBUF slot
            t12 = pool.tile([R, 12 * W], f16, tag=f"sq{c}")
            nc.vector.tensor_add(
                t12[:, :], kw[:, : 12 * W], kw[:, 12 * W : 24 * W]
            )
            t6 = pool.tile([R, 6 * W], f16, tag="t6", bufs=2)
            nc.vector.tensor_add(t6[:, :], t12[:, : 6 * W], t12[:, 6 * W :])
            t3 = pool.tile([R, 3 * W], f16, tag="t3", bufs=2)
            nc.vector.tensor_add(t3[:, :], t6[:, : 3 * W], t6[:, 3 * W :])
            norm = pool.tile([R, W], f16, tag="norm", bufs=2)
            nc.vector.tensor_add(norm[:, :], t3[:, :W], t3[:, W : 2 * W])
            nc.vector.tensor_add(norm[:, :], norm[:, :], t3[:, 2 * W : 3 * W])

            rnorm = pool.tile([R, W], f32, tag="rnorm", bufs=2)
            nc.vector.reciprocal(rnorm[:, :], norm[:, :])

            ot = pool.tile([R, NCL * W], f16, tag="out", bufs=2)
            rb = rnorm[:, :].unsqueeze(1).broadcast_to([R, NCL, W])
            nc.vector.tensor_mul(
                ot[:, :].rearrange("p (n w) -> p n w", n=NCL), accv, rb
            )
            nc.sync.dma_start(
                out_ap[:, c * NCL * W : (c + 1) * NCL * W], ot[:, :]
            )


def _build_program():
    import concourse.bacc as bacc
    import concourse.mybir as mybir
    from concourse import tile

    f16 = mybir.dt.float16

    nc = bacc.Bacc("TRN2", num_devices=8, debug=False)
    I_in = nc.dram_tensor("i_in", [RH, IW], f16, kind="ExternalInput")
    Q_in = nc.dram_tensor("q_in", [RH, QW], f16, kind="ExternalInput")
    OUT = nc.dram_tensor("out", [R, C * NCL * W], f16, kind="ExternalOutput")

    with tile.TileContext(nc) as tc:
        _emit(tc, I_in.ap(), Q_in.ap(), OUT.ap())

    nc.compile()
    return nc


def _get_program():
    if "nc" not in _CACHE:
        _CACHE["nc"] = _build_program()
    return _CACHE["nc"]


def _shard_inputs(Q, I):
    """Host prep: pad, cast fp16, per-shard (rows, chan*cols) layout."""
    Qp = np.pad(
        np.asarray(Q, np.float32), ((0, 0), (0, 0), (PAD, PAD), (PAD, PAD))
    ).astype(np.float16)
    Ip = np.pad(
        np.asarray(I, np.float32), ((0, 0), (0, 0), (PAD, PAD), (PAD, PAD))
    ).astype(np.float16)
    in_maps = []
    for b in range(B):
        for s in range(NSLAB):
            r0 = s * R
            i_sh = Ip[b, :, r0 : r0 + RH, :]  # (C, RH, WP)
            q_sh = Qp[b, :, r0 : r0 + RH, :]  # (NCL, RH, WP)
            in_maps.append(
                {
                    "i_in": np.ascontiguousarray(
                        i_sh.transpose(1, 0, 2).reshape(RH, IW)
                    ),
                    "q_in": np.ascontiguousarray(
                        q_sh.transpose(1, 0, 2).reshape(RH, QW)
                    ),
                }
            )
    return in_maps


def _assemble(outs):
    # outs: list of 8 arrays (R, C*NCL*W), core order = (b, slab)
    o = np.stack([np.asarray(x) for x in outs]).astype(np.float32)
    o = o.reshape(B, NSLAB, R, C, NCL, W)
    o = o.transpose(0, 3, 4, 1, 2, 5).reshape(B, C, NCL, H, W)
    return o


def run(Q, I, trace=False):
    from concourse.bass_utils import run_bass_kernel_spmd

    nc = _get_program()
    in_maps = _shard_inputs(Q, I)
    res = run_bass_kernel_spmd(nc, in_maps, list(range(8)), trace=trace)
    out = _assemble([res.results[i]["out"] for i in range(8)])
    return out, res


def kernel(Q, I):
    out, _ = run(Q, I)
    return out


# revision 21
# speedup vs baseline: 1.1448x; 1.1448x over previous
"""Bilateral filter (nn_BilateralFilter) Trainium2 Bass kernel.

Reference semantics (KERNEL_SIZE=5, THETA_ALPHA=2.0, THETA_BETA=0.1):
    w_k   = exp(-(dx^2+dy^2)/8)                      (24 offsets, center dropped)
    Ki    = exp(-50*(I(p+k) - I(p))^2)               per image channel c
    out[c,n,p] = sum_k w_k*Ki[c,k,p]*Q(n,p+k) / sum_k w_k*Ki[c,k,p]

Sharding: 8 cores = 2 batches x 4 row-slabs of 80 output rows, each slab
shipped with a 2-row halo and 2-col zero padding (84 x 324 per channel).
Device layout: partitions = image rows, free dim = (channel, column).
fp16 on-chip (DVE 2x mode); exponent computed via ACT (Square then Exp with
the spatial weight folded into the exp bias, plus a +8 exponent shift to
keep fp16 sums well inside normal range; the shift cancels in the final
division).  A column-shifted copy of I and Q keeps every access 4-byte
aligned so the DVE stays in its accelerated mode for odd column offsets.
Compute-engine SBUF accesses spanning >32 partitions must start at
partition 0, so each row shift dr gets its own 80-partition copy (5 blocks
packed in one tile, one DMA each for I/Ishift/Q/Qshift).
"""

import math

import numpy as np

B, C, NCL = 2, 3, 6
H = W = 320
KS, PAD = 5, 2
NK = KS * KS - 1          # 24
WP = W + 2 * PAD          # 324
NSLAB = 4
R = H // NSLAB            # 80 output rows per shard
RH = R + 2 * PAD          # 84 rows incl. halo
COEF = 50.0               # 1/(2*theta_beta^2)
SHIFT = 8.0               # exponent shift, cancels in the division
IW = C * WP               # 972
QW = NCL * WP             # 1944

_CACHE: dict = {}


def _offsets():
    return [
        (dr, dc)
        for dr in range(KS)
        for dc in range(KS)
        if not (dr == PAD and dc == PAD)
    ]


def _emit(tc, i_ap, q_ap, out_ap):
    """Emit the per-core program into TileContext tc.

    i_ap:   DRAM AP (RH, C*WP)  fp16
    q_ap:   DRAM AP (RH, NCL*WP) fp16
    out_ap: DRAM AP (R, C*NCL*W) fp16

    Layout: 25 k-slots (dr-major, center included but killed via a -30
    exp bias so it contributes exactly 0), each slot holding (c, x).
    Products are batched over all 5 dc per (c, dr) in one 4-dim-AP op.
    """
    import concourse.bass as bass
    import concourse.mybir as mybir

    f16 = mybir.dt.float16
    f32 = mybir.dt.float32
    AF = mybir.ActivationFunctionType
    nc = tc.nc
    NS = KS * KS           # 25 slots
    CW = C * W             # 960, slot width in d/sq/kw tiles
    CTR = PAD * KS + PAD   # slot 12 = center

    with tc.tile_pool(name="p", bufs=1) as pool:
        # exp biases as per-partition const columns (activation bias AP)
        bias_vals = {}
        for dr in range(KS):
            for dc in range(KS):
                s = dr * KS + dc
                if s == CTR:
                    bias_vals[s] = SHIFT - 30.0
                else:
                    bias_vals[s] = (
                        SHIFT - ((dr - PAD) ** 2 + (dc - PAD) ** 2) / 8.0
                    )
        distinct = sorted(set(bias_vals.values()))
        bcol = {v: j for j, v in enumerate(distinct)}
        bias_t = pool.tile([R, len(distinct)], f32, tag="bias")
        for v, j in bcol.items():
            nc.vector.memset(bias_t[:, j : j + 1], v)

        Ia = pool.tile([R, KS * IW], f16, tag="Ia")
        Qa = pool.tile([R, KS * QW], f16, tag="Qa")
        nc.sync.dma_start(
            Ia[:, :].rearrange("p (dr w) -> p dr w", dr=KS),
            bass.AP(tensor=i_ap.tensor, offset=0,
                    ap=[[IW, R], [IW, KS], [1, IW]]),
        )
        nc.sync.dma_start(
            Qa[:, :].rearrange("p (dr w) -> p dr w", dr=KS),
            bass.AP(tensor=q_ap.tensor, offset=0,
                    ap=[[QW, R], [QW, KS], [1, QW]]),
        )

        def i_v(dr, dc):
            # [R, (c,320)] view of I at offset (dr, dc); c-stride WP
            return bass.AP(
                tensor=Ia.tensor, offset=Ia.offset + dr * IW + dc,
                ap=[[KS * IW, R], [WP, C], [1, W]],
            )

        # d[slot] = I(p+k) - I(p), all 3 channels per op
        d = pool.tile([R, NS * CW], f16, tag="big0")
        cen = i_v(PAD, PAD)
        for dr in range(KS):
            for dc in range(KS):
                s = dr * KS + dc
                dst = d[:, s * CW : (s + 1) * CW].rearrange(
                    "p (c w) -> p c w", c=C
                )
                if s == CTR:
                    nc.vector.memset(d[:, s * CW : (s + 1) * CW], 0.0)
                else:
                    nc.vector.tensor_sub(dst, i_v(dr, dc), cen)

        sq = pool.tile([R, NS * CW], f16, tag="big1")
        nc.scalar.activation(sq[:, :], d[:, :], AF.Square)

        # kw[slot] = exp(-50*sq + SHIFT + ln w_k); center bias -30 -> 0
        kw = pool.tile([R, NS * CW], f16, tag="big0")
        for s in range(NS):
            j = bcol[bias_vals[s]]
            nc.scalar.activation(
                kw[:, s * CW : (s + 1) * CW],
                sq[:, s * CW : (s + 1) * CW],
                AF.Exp,
                bias=bias_t[:, j : j + 1],
                scale=-COEF,
            )

        for c in range(C):
            acc = pool.tile([R, NCL * W], f16, tag=f"acc{c}")
            first = True
            for dr in range(KS):
                # P5[dc, n, x] = kw[5dr+dc, c, x] * Q[n, p+(dr,dc-2)]
                P5 = pool.tile([R, KS * NCL * W], f16, tag="P5", bufs=1)
                kw_src = bass.AP(
                    tensor=kw.tensor,
                    offset=kw.offset + (dr * KS) * CW + c * W,
                    ap=[[NS * CW, R], [CW, KS], [0, NCL], [1, W]],
                )
                q_src = bass.AP(
                    tensor=Qa.tensor, offset=Qa.offset + dr * QW,
                    ap=[[KS * QW, R], [1, KS], [WP, NCL], [1, W]],
                )
                nc.vector.tensor_mul(
                    P5[:, :].rearrange("p (dc n w) -> p dc n w", dc=KS, n=NCL),
                    kw_src,
                    q_src,
                )
                # fold 5 -> 1: [A+C, B+D] ; + ; + E
                NW = NCL * W
                s1 = pool.tile([R, 2 * NW], f16, tag="s1", bufs=1)
                nc.vector.tensor_add(
                    s1[:, :], P5[:, : 2 * NW], P5[:, 2 * NW : 4 * NW]
                )
                s2 = pool.tile([R, NW], f16, tag="s2", bufs=2)
                nc.vector.tensor_add(s2[:, :], s1[:, :NW], s1[:, NW:])
                if first:
                    nc.vector.tensor_add(acc[:, :], s2[:, :], P5[:, 4 * NW :])
                    first = False
                else:
                    s3 = pool.tile([R, NW], f16, tag="s3", bufs=2)
                    nc.vector.tensor_add(s3[:, :], s2[:, :], P5[:, 4 * NW :])
                    nc.vector.tensor_add(acc[:, :], acc[:, :], s3[:, :])

            # norm_c = sum_slots kw[slot, c]: fold 25 slots (center is 0)
            kwc = bass.AP(
                tensor=kw.tensor, offset=kw.offset + c * W,
                ap=[[NS * CW, R], [CW, NS], [1, W]],
            )
            # t12 reuses sq's (dead) big slot
            t12 = pool.tile([R, 12 * W], f16, tag="big1")
            nc.vector.tensor_add(
                t12[:, :].rearrange("p (s w) -> p s w", s=12),
                kwc[:, 0:12],
                kwc[:, 12:24],
            )
            t6 = pool.tile([R, 6 * W], f16, tag="t6", bufs=2)
            nc.vector.tensor_add(t6[:, :], t12[:, : 6 * W], t12[:, 6 * W :])
            t3 = pool.tile([R, 3 * W], f16, tag="t3", bufs=2)
            nc.vector.tensor_add(t3[:, :], t6[:, : 3 * W], t6[:, 3 * W :])
            n1 = pool.tile([R, W], f16, tag="n1", bufs=2)
            nc.vector.tensor_add(n1[:, :], t3[:, :W], t3[:, W : 2 * W])
            nc.vector.tensor_add(n1[:, :], n1[:, :], t3[:, 2 * W : 3 * W])
            norm = pool.tile([R, W], f32, tag="norm", bufs=2)
            nc.vector.tensor_add(norm[:, :], n1[:, :], kwc[:, 24:25].squeeze(1))

            rnorm = pool.tile([R, W], f32, tag="rnorm", bufs=2)
            nc.vector.reciprocal_approx_fast(rnorm[:, :], norm[:, :])
            rnh = pool.tile([R, W], f16, tag="rnh", bufs=2)
            nc.vector.tensor_copy(rnh[:, :], rnorm[:, :])

            ot = pool.tile([R, NCL * W], f16, tag="out", bufs=1)
            rb = rnh[:, :].unsqueeze(1).broadcast_to([R, NCL, W])
            nc.vector.tensor_mul(
                ot[:, :].rearrange("p (n w) -> p n w", n=NCL),
                acc[:, :].rearrange("p (n w) -> p n w", n=NCL),
                rb,
            )
            nc.sync.dma_start(
                out_ap[:, c * NCL * W : (c + 1) * NCL * W], ot[:, :]
            )


def _build_program():
    import concourse.bacc as bacc
    import concourse.mybir as mybir
    from concourse import tile

    f16 = mybir.dt.float16

    nc = bacc.Bacc("TRN2", num_devices=8, debug=False)
    I_in = nc.dram_tensor("i_in", [RH, IW], f16, kind="ExternalInput")
    Q_in = nc.dram_tensor("q_in", [RH, QW], f16, kind="ExternalInput")
    OUT = nc.dram_tensor("out", [R, C * NCL * W], f16, kind="ExternalOutput")

    with tile.TileContext(nc) as tc:
        _emit(tc, I_in.ap(), Q_in.ap(), OUT.ap())

    nc.compile()
    return nc


def _get_program():
    if "nc" not in _CACHE:
        _CACHE["nc"] = _build_program()
    return _CACHE["nc"]


def _shard_inputs(Q, I):
    """Host prep: pad, cast fp16, per-shard (rows, chan*cols) layout."""
    Qp = np.pad(
        np.asarray(Q, np.float32), ((0, 0), (0, 0), (PAD, PAD), (PAD, PAD))
    ).astype(np.float16)
    Ip = np.pad(
        np.asarray(I, np.float32), ((0, 0), (0, 0), (PAD, PAD), (PAD, PAD))
    ).astype(np.float16)
    in_maps = []
    for b in range(B):
        for s in range(NSLAB):
            r0 = s * R
            i_sh = Ip[b, :, r0 : r0 + RH, :]  # (C, RH, WP)
            q_sh = Qp[b, :, r0 : r0 + RH, :]  # (NCL, RH, WP)
            in_maps.append(
                {
                    "i_in": np.ascontiguousarray(
                        i_sh.transpose(1, 0, 2).reshape(RH, IW)
                    ),
                    "q_in": np.ascontiguousarray(
                        q_sh.transpose(1, 0, 2).reshape(RH, QW)
                    ),
                }
            )
    return in_maps


def _assemble(outs):
    # outs: list of 8 arrays (R, C*NCL*W), core order = (b, slab)
    o = np.stack([np.asarray(x) for x in outs]).astype(np.float32)
    o = o.reshape(B, NSLAB, R, C, NCL, W)
    o = o.transpose(0, 3, 4, 1, 2, 5).reshape(B, C, NCL, H, W)
    return o


def run(Q, I, trace=False):
    from concourse.bass_utils import run_bass_kernel_spmd

    nc = _get_program()
    in_maps = _shard_inputs(Q, I)
    res = run_bass_kernel_spmd(nc, in_maps, list(range(8)), trace=trace)
    out = _assemble([res.results[i]["out"] for i in range(8)])
    return out, res


def kernel(Q, I):
    out, _ = run(Q, I)
    return out
